# revision 61
# baseline (speedup 1.0000x reference)
"""ProteinInterfacePrediction fused Bass kernel for 8 TRN2 NeuronCores.

Sharding: core c = (batch b = c//2, L-half h = c%2); each core computes its
(256, 512) output tile. Weights replicated; ligand features sliced with halo.

Decomposition (validated bit-level in numpy vs the jax reference):
  - GNN residual folded into HOPI: pl = Wl@nodeT + (Wl/16)@S, S = sum_k tanh(hn+he)
  - conv1 is rank-separable before relu: conv1(P) = U[co,l] + V[co,r] (+consts),
    boundary columns via mask-augmented 1-D convs, boundary rows via per-core
    flag-baked V-weight variants.
  - conv2 on TensorE: 4-input-row blocks on 128 partitions (K = 4rows x 32ci),
    stride-2 (P/Q dual layouts), 3 dr-taps, 4-way 32-column array tiling.
  - conv3 (1x1) + bias + sigmoid fused at the tail.

I/O is tuned for the axon tunnel (host<->device bytes + per-call program
bytes dominate wall-clock):
  - edges/nodes ship as fp8-e4m3 (converted to bf16 on ScalarE); receptor
    edges are pair-split: each core uploads only its 260-node half-window
    and an on-device pair AllGather stitches the full przA row.
  - shared weights pack into one bf16 blob; each core uploads 1/8th and a
    world AllGather reassembles it. Flag-dependent variants (VWf/VWq,
    VCf/VCq, plmask) and A0W/A511W/W2-row-shift derive on device.
  - the sigmoid output is round-to-nearest uint8 (x*254), dequantized on
    host; conv strips 1-6 run in a hardware For_i loop to shrink the
    program ~2.4x (per-call NEFF handling costs ~27us/instruction).
"""

import numpy as np
import ml_dtypes

B, L, R, KNB = 4, 512, 512, 16
DN, DE = 128, 64
NLIG = 260           # 256 + halo 2 each side
NRECW = 260          # receptor window per core (pair-split)
PLIG = NLIG * KNB
PRECW = NRECW * KNB
NNODE = NLIG + NRECW          # 520 nodes in the combined feature blob
PFEAT = PLIG + PRECW          # 8320 edge positions

# Shared (core-independent) weights, packed row-major into one bf16 blob;
# each core uploads 1/8th and an on-device world AllGather reassembles it.
# A0W/A511W (UW - W1c* taps), W2P1 (partition-shifted W2P0), VWf/VWq and
# VCf/VCq (VW/VC - flag*delta) are derived on device; the per-core 0/1
# flags/masks ride as fp8 columns of the feat blob.
LAY16 = [
    ('WNT', 128, 128), ('WETb', 64, 128),
    ('WlT', 128, 32), ('WlT16', 128, 32), ('WrT', 128, 32), ('WrT16', 128, 32),
    ('UW', 32, 96),
    ('W1c0', 32, 192), ('W1c511', 32, 192),
    ('c0c', 1, 96), ('c511c', 1, 96),
    ('VW', 32, 384), ('VWfd', 32, 96), ('VWqd', 32, 96),
    ('VCfd', 1, 32), ('VCqd', 1, 32), ('VC', 1, 128),
    ('W2P01', 128, 192), ('W3sel', 128, 4),
    ('gnnbias', 128, 1), ('bc2rep', 128, 1), ('b3vec', 128, 1),
]
N16 = sum(r * c for _, r, c in LAY16)
assert N16 % 8 == 0
# Per-core 0/1 constants (rmP0 | rmQ63 | flag0 | flag1), exact in fp8,
# ride as 8 extra columns of the feat blob (two 64-row segments).
PCORE = ['rmP0', 'rmQ63', 'flag0', 'flag1']

_CACHE = {}


def _pack(entries, layout, np_dtype):
    segs = [np.asarray(entries[nm], np.float32).reshape(r * c)
            for nm, r, c in layout]
    return np.ascontiguousarray(
        np.concatenate(segs).astype(np_dtype).reshape(1, -1))


def _host_prep(inputs):
    f32 = np.float32
    f8 = ml_dtypes.float8_e4m3
    bf16 = ml_dtypes.bfloat16
    W1 = np.asarray(inputs['Wc1'], f32)
    W2 = np.asarray(inputs['Wc2'], f32)
    W3 = np.asarray(inputs['Wc3'], f32)[0, :, 0, 0]
    b1 = np.asarray(inputs['bc1'], f32)
    b2 = np.asarray(inputs['bc2'], f32)
    b3 = float(np.asarray(inputs['bc3'], f32)[0])
    Wp = np.asarray(inputs['Wp'], f32)
    bp = np.asarray(inputs['bp'], f32)
    Wl, Wr = Wp[:, :DN], Wp[:, DN:]
    WN = np.asarray(inputs['WN'], f32)
    bN = np.asarray(inputs['bN'], f32)
    WE = np.asarray(inputs['WE'], f32)
    bE = np.asarray(inputs['bE'], f32)

    A = W1.sum(axis=3)
    Wv = W1.sum(axis=2)
    cU = np.einsum('oidr,i->od', W1, bp)

    sh = {}
    sh['WNT'] = WN.T
    sh['WETb'] = WE.T
    sh['gnnbias'] = (bN + bE).reshape(DN, 1)
    sh['WlT'] = Wl.T
    sh['WlT16'] = (Wl / 16.0).T
    sh['WrT'] = Wr.T
    sh['WrT16'] = (Wr / 16.0).T

    def pack3(M):  # (co, ci, dl) -> [32, 96] of [ci, co] blocks
        out = np.zeros((32, 96), f32)
        for dl in range(3):
            out[:, 32 * dl:32 * dl + 32] = M[:, :, dl].T
        return out

    sh['UW'] = pack3(A)

    W1c0 = np.zeros((32, 192), f32)
    W1c511 = np.zeros((32, 192), f32)
    for dl in range(3):
        for t, dr in enumerate((1, 2)):
            W1c0[:, 32 * (2 * dl + t):32 * (2 * dl + t) + 32] = W1[:, :, dl, dr].T
        for t, dr in enumerate((0, 1)):
            W1c511[:, 32 * (2 * dl + t):32 * (2 * dl + t) + 32] = W1[:, :, dl, dr].T
    sh['W1c0'], sh['W1c511'] = W1c0, W1c511

    c0c = np.zeros((1, 96), f32)
    c511c = np.zeros((1, 96), f32)
    for dl in range(3):
        c0c[0, 32 * dl:32 * dl + 32] = np.einsum('oid,i->o', W1[:, :, dl, 1:], bp)
        c511c[0, 32 * dl:32 * dl + 32] = np.einsum('oid,i->o', W1[:, :, dl, :2], bp)
    c0c[0, 32:64] += b1
    c511c[0, 32:64] += b1
    sh['c0c'], sh['c511c'] = c0c, c511c

    # [128, 192]: per dr a 64-wide [P-block | Q-block] pair of 32-out-ch
    # weight sets; P uses rows j=0..2, Q (row-shifted) uses j=1..3.
    W2P01 = np.zeros((128, 192), f32)
    for dr in range(3):
        for j in range(3):
            W2P01[32 * j:32 * j + 32, 64 * dr:64 * dr + 32] = W2[:, :, j, dr].T
            W2P01[32 * (j + 1):32 * (j + 1) + 32,
                  64 * dr + 32:64 * dr + 64] = W2[:, :, j, dr].T
    sh['W2P01'] = W2P01

    W3sel = np.zeros((128, 4), f32)
    for j in range(4):
        W3sel[32 * j:32 * j + 32, j] = W3
    sh['W3sel'] = W3sel
    sh['bc2rep'] = np.tile(b2, 4).reshape(128, 1)
    sh['b3vec'] = np.full((128, 1), b3, f32)

    VW = np.zeros((32, 384), f32)
    for dr in range(3):
        blk = Wv[:, :, dr].T
        for j in range(4):
            VW[:, 128 * dr + 32 * j:128 * dr + 32 * j + 32] = blk
    VWfd = np.zeros((32, 96), f32)
    VWqd = np.zeros((32, 96), f32)
    for dr in range(3):
        VWfd[:, 32 * dr:32 * dr + 32] = W1[:, :, 0, dr].T
        VWqd[:, 32 * dr:32 * dr + 32] = W1[:, :, 2, dr].T
    sh['VW'], sh['VWfd'], sh['VWqd'] = VW, VWfd, VWqd
    vc = cU.sum(axis=1) + b1
    sh['VC'] = np.tile(vc, 4).reshape(1, 128)
    sh['VCfd'] = cU[:, 0].reshape(1, 32)
    sh['VCqd'] = cU[:, 2].reshape(1, 32)

    wb16_full = _pack(sh, LAY16, bf16)
    shard = N16 // 8

    lig_nf = np.asarray(inputs['ligand_node_features'], f32)
    lig_ef = np.asarray(inputs['ligand_edge_features'], f32)
    rec_nf = np.asarray(inputs['receptor_node_features'], f32)
    rec_ef = np.asarray(inputs['receptor_edge_features'], f32)

    maps = []
    for core in range(8):
        b, h = core // 2, core % 2
        lo = 256 * h - 2
        pc = {}

        def window(nf, ef):
            node = np.zeros((NLIG, DN), f32)
            edge = np.zeros((NLIG, KNB, DE), f32)
            g0, g1 = max(lo, 0), min(lo + 260, L)
            node[g0 - lo:g1 - lo] = nf[b, g0:g1]
            edge[g0 - lo:g1 - lo] = ef[b, g0:g1]
            return node, edge

        lig_node, lig_edge = window(lig_nf, lig_ef)
        rec_node, rec_edge = window(rec_nf, rec_ef)

        pc['flag0'] = np.full((128, 1), 1.0 if h == 0 else 0.0, f32)
        pc['flag1'] = np.full((128, 1), 1.0 if h == 1 else 0.0, f32)

        rmP0 = np.ones((128, 1), f32)
        rmQ63 = np.ones((128, 1), f32)
        for j in range(4):
            if not (0 <= 256 * h + (j - 1) < L):
                rmP0[32 * j:32 * j + 32] = 0.0
            if not (0 <= 256 * h + (253 + j) < L):
                rmQ63[32 * j:32 * j + 32] = 0.0
        pc['rmP0'], pc['rmQ63'] = rmP0, rmQ63

        nodes = np.ascontiguousarray(
            np.concatenate([lig_node.T, rec_node.T], axis=1)).astype(f8)
        percore = np.concatenate([pc[nm] for nm in PCORE], axis=1).astype(f8)
        m = {
            # one fp8 blob: ligand edges | receptor edges | node features
            # (nodes [128, 520] as two [64, 520] partition-half segments)
            # | per-core 0/1 consts ([128, 4] as two [64, 4] segments)
            'feat': np.ascontiguousarray(np.concatenate(
                [lig_edge.reshape(PLIG, DE).T.astype(f8),
                 rec_edge.reshape(PRECW, DE).T.astype(f8),
                 nodes[0:64, :], nodes[64:128, :],
                 percore[0:64, :], percore[64:128, :]], axis=1)),
            'wb16': np.ascontiguousarray(
                wb16_full[:, core * shard:(core + 1) * shard]),
        }
        maps.append(m)
    return maps


def _build_program():
    import concourse.bacc as bacc
    import concourse.mybir as mybir
    from concourse.bass import ds
    from concourse.expressions import smax, smin
    from concourse.tile import TileContext

    dt = mybir.dt
    f32, bf16, f8 = dt.float32, dt.bfloat16, dt.float8e4
    AF = mybir.ActivationFunctionType
    ALU = mybir.AluOpType

    nc = bacc.Bacc("TRN2", target_bir_lowering=False, debug=False, num_devices=8)

    feat_d = nc.dram_tensor("feat", [64, PFEAT + 2 * NNODE + 8], f8,
                            kind="ExternalInput")
    wb16_d = nc.dram_tensor("wb16", [1, N16 // 8], bf16, kind="ExternalInput")
    # strip-major layout: row = 512*k + 128*u + p, col = strip-row i;
    # keeps the For_i output DMA offset on the leading dim.
    out = nc.dram_tensor("out", [4096, 32], dt.uint8, kind="ExternalOutput")

    with TileContext(nc) as tc:
        with tc.tile_pool(name="const", bufs=1) as cpool, \
             tc.tile_pool(name="dram", bufs=1, space="DRAM") as dpool:
            W = {}

            # Reassemble the shared weight blob: every core uploads 1/8th,
            # world AllGather concatenates the shards in rank order.
            wbsh = dpool.tile([1, N16 // 8], bf16, tag="wbsh")
            wbfull = dpool.tile([1, N16], bf16, tag="wbfull")
            nc.gpsimd.dma_start(wbsh[:], wb16_d[:])
            nc.gpsimd.collective_compute(
                "AllGather", mybir.AluOpType.bypass,
                replica_groups=[[0, 1, 2, 3, 4, 5, 6, 7]],
                ins=[wbsh.opt()], outs=[wbfull.opt()])

            def load_blob(layout, blob, dtype):
                off = 0
                for nm, r, c in layout:
                    t = cpool.tile([128, c], dtype, tag=f"w_{nm}")
                    nc.sync.dma_start(
                        out=t[0:r, 0:c],
                        in_=blob[0:1, off:off + r * c].rearrange(
                            "o (r c) -> (o r) c", r=r))
                    W[nm] = t
                    off += r * c

            load_blob(LAY16, wbfull, bf16)
            # biases need f32 tiles (ACT bias operands); convert from bf16.
            for nm in ('gnnbias', 'bc2rep', 'b3vec'):
                t = cpool.tile([128, 1], f32, tag=f"wf_{nm}")
                nc.scalar.activation(t[:], W[nm][:], AF.Copy)
                W[nm] = t

            nodes8_s = cpool.tile([128, NNODE], f8, tag="nodes8")
            base = PFEAT + 2 * NNODE
            wcore8 = cpool.tile([128, 4], f8, tag="wcore8")
            for a in range(2):
                nc.sync.dma_start(
                    out=nodes8_s[64 * a:64 * a + 64, :],
                    in_=feat_d[:, PFEAT + NNODE * a:PFEAT + NNODE * (a + 1)])
                nc.sync.dma_start(
                    out=wcore8[64 * a:64 * a + 64, :],
                    in_=feat_d[:, base + 4 * a:base + 4 * (a + 1)])
            nodes_s = cpool.tile([128, NNODE], bf16, tag="nodes")
            nc.scalar.activation(nodes_s[:], nodes8_s[:], AF.Copy)
            for j, nm in enumerate(PCORE):
                t = cpool.tile([128, 1], f32, tag=f"wf_{nm}")
                nc.scalar.activation(t[:], wcore8[:, j:j + 1], AF.Copy)
                W[nm] = t

            # VWf/VWq = VW - flag*delta on the j=1 / j=2 blocks.
            vtmp = cpool.tile([128, 96], bf16, tag="vtmp")
            for dnm, fl, j, tag in (('VWfd', 'flag0', 1, 'w_VWf'),
                                    ('VWqd', 'flag1', 2, 'w_VWq')):
                t = cpool.tile([128, 384], bf16, tag=tag)
                nc.vector.tensor_copy(t[0:32, :], W['VW'][0:32, :])
                nc.vector.tensor_scalar_mul(vtmp[0:32, :], W[dnm][0:32, :],
                                            W[fl][0:32, 0:1])
                tv = t[0:32, :].rearrange("p (d j c) -> p d j c", j=4, c=32)
                vv = W['VW'][0:32, :].rearrange("p (d j c) -> p d j c", j=4, c=32)
                dv = vtmp[0:32, :].rearrange("p (d c) -> p d c", c=32)
                nc.vector.tensor_sub(tv[:, :, j, :], vv[:, :, j, :], dv)
                W[tag[2:]] = t
            # plmask: ones, minus flag0 on window cols 0..1 (l = -2, -1) and
            # flag1 on cols 258..259 (l = 512, 513).
            plm = cpool.tile([128, 260], f32, tag="w_plmask")
            nc.vector.memset(plm[0:32, :], 1.0)
            nc.vector.tensor_scalar_sub(plm[0:32, 0:2], plm[0:32, 0:2],
                                        W['flag0'][0:32, 0:1])
            nc.vector.tensor_scalar_sub(plm[0:32, 258:260], plm[0:32, 258:260],
                                        W['flag1'][0:32, 0:1])
            W['plmask'] = plm
            # VCf/VCq = VC - flag*delta on cols 32:64 / 64:96.
            for dnm, fl, c0, tag in (('VCfd', 'flag0', 32, 'w_VCf'),
                                     ('VCqd', 'flag1', 64, 'w_VCq')):
                t = cpool.tile([128, 128], bf16, tag=tag)
                nc.vector.tensor_copy(t[0:1, :], W['VC'][0:1, :])
                nc.vector.tensor_scalar_mul(vtmp[0:1, 0:32], W[dnm][0:1, :],
                                            W[fl][0:1, 0:1])
                nc.vector.tensor_sub(t[0:1, c0:c0 + 32], W['VC'][0:1, c0:c0 + 32],
                                     vtmp[0:1, 0:32])
                W[tag[2:]] = t

            ONE1_s = cpool.tile([128, 1], bf16, tag="one1")
            nc.vector.memset(ONE1_s[0:1, 0:1], 1.0)
            ONESR_s = cpool.tile([128, 512], bf16, tag="onesr")
            nc.vector.memset(ONESR_s[0:1, :], 1.0)

            S_all = cpool.tile([128, NNODE], bf16)
            S_lig = S_all[:, 0:NLIG]
            S_rec = S_all[:, NLIG:NNODE]
            Stmp = cpool.tile([128, 32], bf16, tag="Stmp")
            plzA = cpool.tile([128, 260], bf16)    # rows 0-31 plz, row 32 mask
            przA = cpool.tile([128, 514], bf16)
            U_sb = cpool.tile([128, 260], f32)
            Uc0_sb = cpool.tile([128, 260], f32)
            Uc511_sb = cpool.tile([128, 260], f32)
            A0AUG = cpool.tile([128, 96], bf16)
            A511AUG = cpool.tile([128, 96], bf16)
            # V selection tables for the unified strip loop: VPt = [rm-masked
            # V_first | V_rep], VQt = [V_rep | rm-masked V_qlast]; the l-edge
            # strips pick their variant via a ds() column offset.
            VPt = cpool.tile([128, 1024], f32, tag="VPt")
            VQt = cpool.tile([128, 1024], f32, tag="VQt")
            uP = cpool.tile([128, 64], f32, tag="uP")
            uQ = cpool.tile([128, 64], f32, tag="uQ")
            uc0P = cpool.tile([128, 64], f32, tag="uc0P")
            uc0Q = cpool.tile([128, 64], f32, tag="uc0Q")
            uc511P = cpool.tile([128, 64], f32, tag="uc511P")
            uc511Q = cpool.tile([128, 64], f32, tag="uc511Q")

            # ================= GNN phase =================
            with tc.tile_pool(name="gnn", bufs=4) as gpool, \
                 tc.tile_pool(name="gpsum", bufs=3, space="PSUM") as gpsum, \
                 tc.tile_pool(name="spsum", bufs=1, space="PSUM") as spsum:

                # 520 nodes as 16 uniform 32-node chunks in a hardware loop
                # plus one static 8-node tail; the chunk grid spans the
                # lig/rec boundary transparently (positions are nodewise).
                def gnn_chunk(cix, cw):
                    dyn = not isinstance(cix, int)
                    pos = cw * KNB
                    e8 = gpool.tile([128, 512], f8, tag="e8")
                    src = (feat_d[:, ds(cix * 512, pos)] if dyn
                           else feat_d[:, cix * 512:cix * 512 + pos])
                    nc.sync.dma_start(out=e8[0:64, 0:pos], in_=src)
                    eb = gpool.tile([128, 512], bf16, tag="eb")
                    nc.scalar.activation(eb[0:64, 0:pos], e8[0:64, 0:pos],
                                         AF.Copy)
                    hz = gpsum.tile([128, 512], f32, tag="hz")
                    nc.tensor.matmul(hz[:, 0:pos], W['WETb'][0:64, :],
                                     eb[0:64, 0:pos], start=True, stop=False)
                    nsl = (nodes_s[:, ds(cix * 32, cw)] if dyn
                           else nodes_s[:, cix * 32:cix * 32 + cw])
                    rhs = nsl.unsqueeze(2).broadcast_to([128, cw, KNB])
                    nc.tensor.matmul(hz[:, 0:pos], W['WNT'][:], rhs,
                                     start=False, stop=True)
                    zt = gpool.tile([128, 512], bf16, tag="zt")
                    nc.scalar.activation(zt[:, 0:pos], hz[:, 0:pos], AF.Tanh,
                                         bias=W['gnnbias'][:, 0:1])
                    ztr = zt[:, 0:pos].rearrange("p (n k) -> p n k", k=KNB)
                    with nc.allow_low_precision(
                            reason="bf16 S output; rel-err gate is 2e-2"):
                        nc.vector.reduce_sum(Stmp[:, 0:cw], ztr,
                                             axis=mybir.AxisListType.X)
                    dstS = (S_all[:, ds(cix * 32, cw)] if dyn
                            else S_all[:, cix * 32:cix * 32 + cw])
                    nc.sync.dma_start(out=dstS, in_=Stmp[:, 0:cw])

                with tc.For_i(0, 16) as cc:
                    gnn_chunk(cc, 32)
                gnn_chunk(16, 8)

                # ---- HOPI ----
                pp = spsum.tile([128, 512], f32, tag="sp")
                nc.tensor.matmul(pp[0:32, 0:NLIG], W['WlT'][0:128, :],
                                 nodes_s[:, 0:NLIG], start=True, stop=False)
                nc.tensor.matmul(pp[0:32, 0:NLIG], W['WlT16'][0:128, :],
                                 S_lig, start=False, stop=True)
                nc.vector.tensor_mul(plzA[0:32, :], pp[0:32, 0:260],
                                     W['plmask'][0:32, :])
                nc.scalar.activation(plzA[32:33, :], W['plmask'][0:1, :],
                                     AF.Copy)

                # Receptor HOPI on the local 288-window; the pair AllGather
                # stitches the two halves into the full 512-wide przA.
                pp2 = spsum.tile([128, 512], f32, tag="sp")
                nc.tensor.matmul(pp2[0:32, 0:NRECW], W['WrT'][0:128, :],
                                 nodes_s[:, NLIG:NLIG + NRECW],
                                 start=True, stop=False)
                nc.tensor.matmul(pp2[0:32, 0:NRECW], W['WrT16'][0:128, :],
                                 S_rec, start=False, stop=True)
                przH = cpool.tile([128, 256], bf16, tag="przH")
                nc.scalar.activation(przH[0:32, :], pp2[0:32, 2:258], AF.Copy)
                przH_d = dpool.tile([32, 256], bf16, tag="przH_d")
                przF_d = dpool.tile([64, 256], bf16, tag="przF_d")
                nc.gpsimd.dma_start(przH_d[:], przH[0:32, :])
                nc.gpsimd.collective_compute(
                    "AllGather", mybir.AluOpType.bypass,
                    replica_groups=[[0, 1], [2, 3], [4, 5], [6, 7]],
                    ins=[przH_d.opt()], outs=[przF_d.opt()])
                nc.vector.memset(przA[0:32, 0:1], 0.0)
                nc.vector.memset(przA[0:32, 513:514], 0.0)
                nc.gpsimd.dma_start(przA[0:32, 1:257], przF_d[0:32, :])
                nc.gpsimd.dma_start(przA[0:32, 257:513], przF_d[32:64, :])

                # ---- U ----
                up = spsum.tile([128, 512], f32, tag="sp")
                for dl in range(3):
                    nc.tensor.matmul(up[0:32, 0:258],
                                     W['UW'][0:32, 32 * dl:32 * dl + 32],
                                     plzA[0:32, dl:dl + 258],
                                     start=(dl == 0), stop=(dl == 2))
                nc.scalar.activation(U_sb[0:32, 0:258], up[0:32, 0:258], AF.Copy)

                # ---- c0 / c511 rows ----
                # A0W = UW - tap0 = UW - W1c511[t=0]; A511W = UW - tap2 =
                # UW - W1c0[t=1] (per 32-col dl block).
                uwv = W['UW'][0:32, :].rearrange("p (d c) -> p d c", c=32)
                for aug, src, tsel in ((A0AUG, 'W1c511', 0), (A511AUG, 'W1c0', 1)):
                    sv = W[src][0:32, :].rearrange("p (d t c) -> p d t c",
                                                   t=2, c=32)
                    av = aug[0:32, :].rearrange("p (d c) -> p d c", c=32)
                    nc.vector.tensor_sub(av, uwv, sv[:, :, tsel, :])
                for which, (w1nm, ccnm, dst) in enumerate(
                        (('W1c0', 'c0c', A0AUG), ('W1c511', 'c511c', A511AUG))):
                    cp = spsum.tile([128, 512], f32, tag="sp")
                    for dl in range(3):
                        for t in range(2):
                            col = (1 + t) if which == 0 else (511 + t)
                            nc.tensor.matmul(
                                cp[0:1, 32 * dl:32 * dl + 32],
                                przA[0:32, col:col + 1],
                                W[w1nm][0:32, 32 * (2 * dl + t):
                                        32 * (2 * dl + t) + 32],
                                start=(t == 0), stop=False)
                        nc.tensor.matmul(
                            cp[0:1, 32 * dl:32 * dl + 32],
                            ONE1_s[0:1, 0:1],
                            W[ccnm][0:1, 32 * dl:32 * dl + 32],
                            start=False, stop=True)
                    nc.scalar.activation(dst[32:33, 0:96], cp[0:1, 0:96], AF.Copy)

                # ---- Ucol0 / Ucol511 ----
                for AUG, dstu in ((A0AUG, Uc0_sb), (A511AUG, Uc511_sb)):
                    ucp = spsum.tile([128, 512], f32, tag="sp")
                    for dl in range(3):
                        nc.tensor.matmul(ucp[0:32, 0:258],
                                         AUG[0:33, 32 * dl:32 * dl + 32],
                                         plzA[0:33, dl:dl + 258],
                                         start=(dl == 0), stop=(dl == 2))
                    nc.scalar.activation(dstu[0:32, 0:258], ucp[0:32, 0:258],
                                         AF.Copy)

                # ---- V variants (written straight into the tables) ----
                for vwnm, vcnm, vt in (('VW', 'VC', VPt[:, 512:1024]),
                                       ('VWf', 'VCf', VPt[:, 0:512]),
                                       ('VWq', 'VCq', VQt[:, 512:1024])):
                    vp = spsum.tile([128, 512], f32, tag="sp")
                    for dr in range(3):
                        nc.tensor.matmul(vp[:, 0:512],
                                         W[vwnm][0:32, 128 * dr:128 * dr + 128],
                                         przA[0:32, dr:dr + 512],
                                         start=(dr == 0), stop=False)
                    nc.tensor.matmul(vp[:, 0:512], W[vcnm][0:1, :],
                                     ONESR_s[0:1, :], start=False, stop=True)
                    nc.scalar.activation(vt, vp[:, 0:512], AF.Copy)
                nc.vector.tensor_copy(VQt[:, 0:512], VPt[:, 512:1024])
                nc.vector.tensor_scalar_mul(VPt[:, 0:512], VPt[:, 0:512],
                                            W['rmP0'][:, 0:1])
                nc.vector.tensor_scalar_mul(VQt[:, 512:1024], VQt[:, 512:1024],
                                            W['rmQ63'][:, 0:1])

                # ---- u relayouts (i = 4s+j for P, 4s+2+j for Q) ----
                for (src, dstP, dstQ) in ((U_sb, uP, uQ), (Uc0_sb, uc0P, uc0Q),
                                          (Uc511_sb, uc511P, uc511Q)):
                    srcr = src[0:32, 0:260].rearrange("c (s f) -> c s f", f=4)
                    for j in range(4):
                        nc.sync.dma_start(out=dstP[32 * j:32 * j + 32, 0:64],
                                          in_=srcr[:, 0:64, j])
                    for j in range(2):
                        nc.sync.dma_start(out=dstQ[32 * j:32 * j + 32, 0:64],
                                          in_=srcr[:, 0:64, 2 + j])
                    for j in range(2, 4):
                        nc.sync.dma_start(out=dstQ[32 * j:32 * j + 32, 0:64],
                                          in_=srcr[:, 1:65, j - 2])
                for (t, col, rm) in ((uP, 0, W['rmP0']), (uc0P, 0, W['rmP0']),
                                     (uc511P, 0, W['rmP0']), (uQ, 63, W['rmQ63']),
                                     (uc0Q, 63, W['rmQ63']),
                                     (uc511Q, 63, W['rmQ63'])):
                    nc.vector.tensor_mul(t[:, col:col + 1], t[:, col:col + 1],
                                         rm[:])

            # ================= conv pipeline =================
            # Strips 0 and 7 (which touch the l-boundary rows) are unrolled;
            # the 6 interior strips run in one hardware For_i loop with
            # register-offset (ds) slices, shrinking the program ~4x.
            with tc.tile_pool(name="x1", bufs=3) as x1pool, \
                 tc.tile_pool(name="x2", bufs=3) as x2pool, \
                 tc.tile_pool(name="osb", bufs=2) as opool, \
                 tc.tile_pool(name="cpsum", bufs=4, space="PSUM") as cpsum, \
                 tc.tile_pool(name="c3ps", bufs=2, space="PSUM") as c3psum:

                def conv_strip(kx):
                    def col(t, mult, off, size):
                        return t[:, ds(kx * mult + off, size)]

                    x1P = x1pool.tile([128, 8 * 514], bf16, tag="x1P")
                    x1Q = x1pool.tile([128, 8 * 514], bf16, tag="x1Q")
                    for s in range(8):
                        for (tile_, uu, Vgen) in (
                                (x1P, uP,
                                 VPt[:, ds(smin(kx, 1) * 512, 512)] if s == 0
                                 else VPt[:, 512:1024]),
                                (x1Q, uQ,
                                 VQt[:, ds(smax(kx - 6, 0) * 512, 512)] if s == 7
                                 else VPt[:, 512:1024])):
                            dst = tile_[:, s * 514 + 1:s * 514 + 513]
                            bias_ap = col(uu, 8, s, 1)
                            # all on DVE: ACT ignores register-offset bias APs
                            # inside the hardware loop.
                            nc.vector.tensor_scalar(dst, Vgen, bias_ap,
                                                    0.0, ALU.add, ALU.max)
                    for tile_, ucol0, ucol511 in ((x1P, uc0P, uc511P),
                                                  (x1Q, uc0Q, uc511Q)):
                        tr = tile_[:].rearrange("p (s c) -> p s c", c=514)
                        nc.vector.memset(tr[:, :, 0], 0.0)
                        nc.vector.memset(tr[:, :, 513], 0.0)
                        nc.vector.tensor_scalar(tr[:, :, 1], col(ucol0, 8, 0, 8),
                                                0.0, None, ALU.max)
                        nc.vector.tensor_scalar(tr[:, :, 512],
                                                col(ucol511, 8, 0, 8),
                                                0.0, None, ALU.max)

                    x2 = x2pool.tile([128, 8 * 512], bf16, tag="x2")
                    for s in range(8):
                        c2 = cpsum.tile([128, 512], f32, tag="c2")
                        for dr in range(3):
                            w = W['W2P01'][:, 64 * dr:64 * dr + 64]
                            rhsP = x1P[:, s * 514 + dr:s * 514 + dr + 512]
                            rhsQ = x1Q[:, s * 514 + dr:s * 514 + dr + 512]
                            st, sp_ = (dr == 0), (dr == 2)
                            nc.tensor.matmul(c2[0:64, :], w, rhsP, start=st,
                                             stop=sp_, tile_position=(0, 0),
                                             skip_group_check=True)
                            nc.tensor.matmul(c2[64:128, :], w, rhsQ, start=st,
                                             stop=sp_, tile_position=(0, 64),
                                             skip_group_check=True)
                        dst2 = x2[:, s * 512:(s + 1) * 512]
                        if s % 3 != 2:
                            nc.scalar.activation(dst2, c2[:], AF.Relu,
                                                 bias=W['bc2rep'][:, 0:1])
                        else:
                            nc.vector.tensor_scalar(dst2, c2[:],
                                                    W['bc2rep'][:, 0:1], 0.0,
                                                    ALU.add, ALU.max)

                    # conv3: logits transposed onto 128 partitions (r-slab on
                    # partitions, strip-row on free); undone host-side.
                    c3p = c3psum.tile([128, 128], f32, tag="c3")
                    for s in range(8):
                        xc = x2[:, s * 512:(s + 1) * 512]
                        for u in range(4):
                            nc.tensor.matmul(
                                c3p[:, 32 * u + 4 * s:32 * u + 4 * s + 4],
                                xc[:, 128 * u:128 * u + 128],
                                W['W3sel'][:, 0:4], start=True, stop=True)
                    osb = opool.tile([128, 128], f32, tag="osb")
                    nc.scalar.activation(osb[:], c3p[:], AF.Sigmoid,
                                         bias=W['b3vec'][:, 0:1])
                    osb8 = opool.tile([128, 128], dt.uint8, tag="osb8")
                    nc.vector.tensor_scalar(osb8[:], osb[:], 254.0, 0.0,
                                            ALU.mult, ALU.add)
                    # osb8[p, 32u+4s+m] = q(sigmoid(logit[row=4s+m, r=128u+p]))
                    osr = osb8[:].rearrange("p (u c) -> p u c", c=32)
                    dstd = out[ds(kx * 512, 512), :].rearrange(
                        "(u p) i -> p u i", p=128)
                    nc.sync.dma_start(out=dstd, in_=osr)

                with tc.For_i(0, 8) as kk:
                    conv_strip(kk)

    nc.compile()
    return nc


def kernel(**inputs):
    from concourse.bass_utils import run_bass_kernel_spmd
    if "nc" not in _CACHE:
        _CACHE["nc"] = _build_program()
    nc = _CACHE["nc"]
    maps = _host_prep(inputs)
    res = run_bass_kernel_spmd(nc, maps, core_ids=list(range(8)))
    _CACHE["last_result"] = res
    full = np.zeros((B, L, R), np.float32)
    for core in range(8):
        b, h = core // 2, core % 2
        arr = res.results[core]["out"].reshape(8, 4, 128, 32)
        q = arr.transpose(1, 2, 0, 3).reshape(512, 256).astype(np.float32) / 254.0
        full[b, 256 * h:256 * h + 256, :] = q.T
    return full
